# revision 19
# baseline (speedup 1.0000x reference)
"""Trainium2 Bass kernel for ColorEntropyLoss.

Math (per batch b, attention map s):
    color_dist[b,s,c] = sum_h attn[b,s,h] * (grid[b,h] == c)       # 10-bin weighted histogram
    p = color_dist / (T := sum_c color_dist + 1e-8)
    entropy[b,s]      = -sum_c p * log(p + 1e-8)
                      = log(T) - (sum_c cd*log(cd + eps)) / T      # (zeros inert)
    out               = mean(entropy)

Sharding: pure data parallelism over batch B=512 across 8 NeuronCores
(64 batches/core), 8 groups of 8 batches per core; a group packs 128
SBUF partitions as (8 batches x 16 maps).

Layout: the host hands attn already in "contraction-major" order —
per group a [128, 4096] f32 slab whose partition dim is pixel-in-chunk
and whose columns are chunk-major x (b,s) — so the histogram is a
straight PSUM-accumulated bf16 matmul chain with NO on-device
transposes:
    out[(b,s),(c,b')] += attnT_chunk.T @ onehot_chunk
The grid arrives host-transposed as bf16 [128, 32*64]; one broadcast
is_equal per group builds all 32 one-hot chunks (all 8 emitted before
any PSUM-gated vector work so mask building never stalls the PE).
The two tiny constants (color iota, block-diag selector) also come
from the host, so the gpsimd engine runs NOTHING but SWDGE
descriptor-gen (iota/affine_select would force ucode lib swaps and
queue behind ring-throttled desc-gens — that cost 8us in one rev).

DMA plan (per-core HBM read caps at ~432 B/ns; attn is 16.8 MB f32).
The SWDGE ring takes ~1-4.5us to arm after the first descriptor and
then sustains ~216 B/ns write-side; the two HWDGE queues (sync/scalar,
~0.8us arm, ~100-150 B/ns each) carry only the early small tensors
(gridT halves, constants) plus one staged f32 quarter of group 0, all
done before SWDGE reaches full rate. SWDGE cast-DMA (f32 HBM read,
bf16 SBUF write) carries the rest: g0's other quarters first, g1..g6
whole, g7 in shrinking pieces (1024/1024/1024/512/256/256 cols) so
compute trails the last bytes closely. The mean over the 8192
per-(b,s) entropies is done host-side (the "cheap all-reduce" from
the sharding hint).
"""

import numpy as np
from contextlib import ExitStack

NUM_COLORS = 10
EPS = 1e-8
B, S, H, W = 512, 16, 64, 64
HW = H * W                      # 4096
N_CORES = 8
B_PER_CORE = B // N_CORES       # 64
N_GROUPS = 8                    # groups per core
B_PER_GROUP = B_PER_CORE // N_GROUPS  # 8 batches -> 128 partitions
P = 128
CHUNK = 128
N_CHUNKS = HW // CHUNK          # 32
NC80 = B_PER_GROUP * NUM_COLORS  # 80
QTR = HW // 4                   # 1024 cols = 8 chunks
GCOL = N_CHUNKS * B_PER_CORE    # 2048 gridT cols

_CACHE = {}


def _build_nc():
    import concourse.bacc as bacc
    import concourse.tile as tile
    import concourse.bass as bass
    from concourse import mybir

    f32 = mybir.dt.float32
    bf16 = mybir.dt.bfloat16
    OP = mybir.AluOpType
    AF = mybir.ActivationFunctionType

    nc = bacc.Bacc(
        "TRN2", target_bir_lowering=False, debug=False, num_devices=N_CORES
    )

    # row = g*128 + p (p = pixel-in-chunk), col = k*128 + b'*16 + s
    attn_in = nc.dram_tensor(
        "attn_in", [N_GROUPS * P, HW], f32, kind="ExternalInput"
    ).ap()
    # row = p, col = k*64 + b   (b = batch within core)
    grid_in = nc.dram_tensor(
        "grid_in", [P, GCOL], bf16, kind="ExternalInput"
    ).ap()
    # cb_in[p, c*8+b'] = c ; bd_in[p, c*8+b'] = (b' == p//16)
    cb_in = nc.dram_tensor("cb_in", [P, NC80], bf16, kind="ExternalInput").ap()
    bd_in = nc.dram_tensor("bd_in", [P, NC80], f32, kind="ExternalInput").ap()
    # [p=(b,s), g] = +sum_c p*ln(p) for group g; host negates & averages.
    ent_out = nc.dram_tensor(
        "ent_out", [P, N_GROUPS], f32, kind="ExternalOutput"
    ).ap()

    with tile.TileContext(nc) as tc:
        with ExitStack() as ctx:
            singles = ctx.enter_context(tc.tile_pool(name="singles", bufs=1))
            pool_s = ctx.enter_context(tc.tile_pool(name="pool_s", bufs=3))
            psum_cd = ctx.enter_context(
                tc.tile_pool(name="psum_cd", bufs=4, space="PSUM")
            )

            # ---- HWDGE: gridT halves split across the two queues, the
            # tiny constants, then one staged f32 quarter of group 0.
            # All done before SWDGE reaches full rate. ----
            gridT = singles.tile([P, GCOL], bf16)
            const_cb = singles.tile([P, NC80], bf16)
            mask_bd = singles.tile([P, NC80], f32)
            # Only the two tiny constants ride the HWDGE queues (they
            # run at a miserly ~50-110 B/ns and SUPPRESS the SWDGE
            # stream while active — every bulk byte belongs on SWDGE).
            nc.sync.dma_start(out=const_cb, in_=cb_in)
            nc.scalar.dma_start(out=mask_bd, in_=bd_in)

            # ---- SWDGE burst: gpsimd runs ONLY desc-gens, in priority
            # order: gridT (masks derive from it), then group 7's LAST
            # quarter (cast-DMA straight into its tile, so the tail
            # matmuls have their final chunks resident early), then the
            # bulk groups, ending with g7's other quarters for a
            # low-latency drain. ----
            attn_tiles = [
                singles.tile([P, HW], bf16, name=f"attnT{g}")
                for g in range(N_GROUPS)
            ]
            g7 = attn_in[7 * P : 8 * P, :]
            nc.gpsimd.dma_start(out=gridT, in_=grid_in)
            nc.gpsimd.dma_start(
                out=attn_tiles[7][:, 3 * QTR : HW], in_=g7[:, 3 * QTR : HW]
            )
            for g in range(N_GROUPS - 1):
                nc.gpsimd.dma_start(
                    out=attn_tiles[g], in_=attn_in[g * P : (g + 1) * P, :]
                )
            for lo, hi in ((0, 1024), (1024, 2048), (2048, 2560), (2560, 3072)):
                nc.gpsimd.dma_start(
                    out=attn_tiles[7][:, lo:hi], in_=g7[:, lo:hi]
                )

            eps_tile = singles.tile([P, 1], f32)
            nc.vector.memset(eps_tile, EPS)
            ent_sb = singles.tile([P, N_GROUPS], f32)

            # ---- all one-hot masks up front: one strided is_equal per
            # group; mask flat [128, 2560]: col = k*80 + c*8 + b ----
            masks = []
            for g in range(N_GROUPS):
                mask = singles.tile(
                    [P, N_CHUNKS * NC80], bf16, name=f"mask{g}"
                )
                gT = gridT[:, :]
                in0 = bass.AP(
                    tensor=gT.tensor,
                    offset=gT.offset + g * B_PER_GROUP,
                    ap=[
                        gT.ap[0],
                        [B_PER_CORE, N_CHUNKS],
                        [0, NUM_COLORS],
                        [1, B_PER_GROUP],
                    ],
                )
                cC = const_cb[:, :]
                in1 = bass.AP(
                    tensor=cC.tensor,
                    offset=cC.offset,
                    ap=[cC.ap[0], [0, N_CHUNKS], [1, NC80]],
                )
                mk = mask[:, :]
                mout = bass.AP(
                    tensor=mk.tensor,
                    offset=mk.offset,
                    ap=[mk.ap[0], [NC80, N_CHUNKS], [1, NC80]],
                )
                nc.vector.tensor_tensor(
                    out=mout, in0=in0, in1=in1, op=OP.is_equal
                )
                masks.append(mask)

            for g in range(N_GROUPS):
                attnT = attn_tiles[g]
                mask = masks[g]

                # ---- histogram: 32 accumulating bf16 matmuls -> PSUM f32.
                # For g7 the chunk order puts the early-resident last
                # quarter before the last-arriving q2, so only 8 matmuls
                # trail the final DMA bytes. ----
                if g == N_GROUPS - 1:
                    korder = (
                        list(range(16)) + list(range(24, 32)) + list(range(16, 24))
                    )  # last-arriving 512-col pieces (chunks 16-23) run last
                else:
                    korder = list(range(N_CHUNKS))
                ps_c = psum_cd.tile([P, NC80], f32, name="ps_c", tag="cd")
                for i, k in enumerate(korder):
                    nc.tensor.matmul(
                        ps_c,
                        attnT[:, k * CHUNK : (k + 1) * CHUNK],
                        mask[:, k * NC80 : (k + 1) * NC80],
                        start=(i == 0),
                        stop=(i == N_CHUNKS - 1),
                    )

                # ---- masked copy to SBUF + row-sum T in one op; the Ln
                # of the raw histogram runs on the scalar engine IN
                # PARALLEL (ps_c >= 0 always; masked-out cols get
                # zeroed by cd before the cd*ln product). ----
                cd = pool_s.tile([P, NC80], f32, name="cd_sb", tag="cd_sb")
                ssum = pool_s.tile([P, 1], f32, name="ssum", tag="ssum")
                lcd = pool_s.tile([P, NC80], f32, name="lcd", tag="lcd")
                nc.scalar.activation(lcd, ps_c, AF.Ln, bias=eps_tile[:, :])
                nc.vector.scalar_tensor_tensor(
                    out=cd,
                    in0=ps_c,
                    scalar=1.0,
                    in1=mask_bd[:, :],
                    op0=OP.mult,
                    op1=OP.mult,
                    accum_out=ssum,
                )

                # ---- entropy: ent = sq/T - ln(T),  sq = sum cd*ln(cd+eps)
                # (host negates).  T ~ 2048 so the +eps on T is dropped. ----
                lnT = pool_s.tile([P, 1], f32, name="lnT", tag="lnT")
                nc.scalar.activation(lnT, ssum, AF.Ln)
                rT = pool_s.tile([P, 1], f32, name="rT", tag="rT")
                nc.vector.reciprocal(rT, ssum)
                q_t = pool_s.tile([P, NC80], f32, name="q_t", tag="q_t")
                sq = pool_s.tile([P, 1], f32, name="sq", tag="sq")
                nc.vector.scalar_tensor_tensor(
                    out=q_t,
                    in0=cd,
                    scalar=1.0,
                    in1=lcd,
                    op0=OP.mult,
                    op1=OP.mult,
                    accum_out=sq,
                )
                nc.vector.scalar_tensor_tensor(
                    out=ent_sb[:, g : g + 1],
                    in0=sq,
                    scalar=rT[:, :],
                    in1=lnT,
                    op0=OP.mult,
                    op1=OP.subtract,
                )

            # Final result rides the WARM SWDGE ring (the cold sync
            # HWDGE queue takes ~1.2us for these 128 small packets).
            nc.gpsimd.dma_start(out=ent_out, in_=ent_sb)

    nc.compile()
    return nc


def _get_nc():
    if "nc" not in _CACHE:
        _CACHE["nc"] = _build_nc()
    return _CACHE["nc"]


def _consts():
    import ml_dtypes

    if "cb" not in _CACHE:
        c = np.arange(NUM_COLORS, dtype=np.float32)  # [10]
        cb = np.broadcast_to(
            c[:, None], (NUM_COLORS, B_PER_GROUP)
        ).reshape(1, NC80)
        _CACHE["cb"] = np.ascontiguousarray(
            np.broadcast_to(cb, (P, NC80))
        ).astype(ml_dtypes.bfloat16)
        rows = np.arange(P) // S                      # p -> b'
        bp = np.tile(np.arange(B_PER_GROUP), NUM_COLORS)  # col -> b'
        _CACHE["bd"] = (
            (bp[None, :] == rows[:, None]).astype(np.float32)
        )
    return _CACHE["cb"], _CACHE["bd"]


def _make_in_maps(attn_weights, grids):
    import ml_dtypes

    attn = np.ascontiguousarray(attn_weights, dtype=np.float32).reshape(
        B, S, HW
    )
    grid = np.asarray(grids)
    cb, bd = _consts()
    in_maps = []
    for c in range(N_CORES):
        lo, hi = c * B_PER_CORE, (c + 1) * B_PER_CORE
        # [64,16,4096] -> [g, b', s, k, p] -> [g, p, k, b', s]
        a5 = attn[lo:hi].reshape(N_GROUPS, B_PER_GROUP, S, N_CHUNKS, CHUNK)
        a_t = np.ascontiguousarray(a5.transpose(0, 4, 3, 1, 2)).reshape(
            N_GROUPS * P, HW
        )
        # [64,4096] -> [b, k, p] -> [p, k, b], values 0..9 exact in bf16
        g3 = grid[lo:hi].reshape(B_PER_CORE, N_CHUNKS, CHUNK)
        g_t = (
            np.ascontiguousarray(g3.transpose(2, 1, 0))
            .astype(np.float32)
            .astype(ml_dtypes.bfloat16)
            .reshape(P, GCOL)
        )
        in_maps.append(
            {"attn_in": a_t, "grid_in": g_t, "cb_in": cb, "bd_in": bd}
        )
    return in_maps


def kernel(attn_weights: np.ndarray, grids: np.ndarray) -> np.ndarray:
    from concourse.bass_utils import run_bass_kernel_spmd

    nc = _get_nc()
    in_maps = _make_in_maps(attn_weights, grids)
    res = run_bass_kernel_spmd(nc, in_maps, core_ids=list(range(N_CORES)))

    total = 0.0
    for c in range(N_CORES):
        total += float(res.results[c]["ent_out"].astype(np.float64).sum())
    return np.float32(-total / (B * S))
